# revision 56
# baseline (speedup 1.0000x reference)
"""Trainium2 Bass kernel for nn_CrossAttentionFusion (v13, mixed precision).

Math (per batch b), all feature-major on device:
    xq = F_VNet[b]      [C=256, N=4096]
    xk = F_Knowledge[b] [32, 4096]
    Scores: S = xk.T @ G with G = wg.T @ xq + bg, wg = SCALE*(Wq.T @ Wk).
    Values are RANK-32: V' = xk.T @ wu with wu = Wv.T @ Wo.T [32, 256]:
        out = (E @ xk.T @ wu) / d,  E = exp(S),  d = E @ 1.
    Device accumulates Z[f, q] = sum_k xkT_aug[k, f] E[k, q] per key tile
    into PSUM rows 0-32; xkT_aug has a 33rd ones-column so row 32 of Z is
    the softmax denominator d.  The bias path is folded into the value
    matmul: wus row 32 = boe (= bo + Wo@bv), so y = wu.T@Z + boe*d and
    y/d = attn_out + boe with no extra vector work.
    d is broadcast across partitions with one select-matmul (sel row 32
    is ones), reciprocal'd on DVE, and out = y*rd + xq (residual).

    S matmuls are 4x row-tiled (K=32 at partitions 32j) on a host-banded
    xk4 layout. G is computed 4-stacked (W4 = tile(wg, 4)).

Mixed precision tuned against the error budget (harness gate 2e-2,
measured absmax/scale ~4.3e-3): queries and the value-side keys (xkt)
and E travel as bf16 (the bulk of the HBM bytes; their rounding mostly
cancels through the softmax normalization), while the score-side keys
(xk4), G, the value weights wus, and the z/denominator copy stay
fp32/f32r -- bf16 score inputs compound to ~7e-3.  The G weights are
bf16 hi+lo halves accumulated in two matmul passes (fp32-accurate wg
for free).  Input DMA drops 2.3MB -> 1.55MB, shortening the DMA-bound
startup and the window where concurrent DMA holds the PE at reduced
HAM duty.  Zero-weight filler matmuls accumulate +0.0 into the live z
group to keep the PE busy through the ACT-bound steady state (the exp
stream on the scalar engine, ~1.18us/step, is the bottleneck); S
matmuls for step i+2 are emitted ahead of the Z pair so the next exp
never waits on them.

Sharding: 8 cores = batch(2) x query-chunk(4 x 1024 tokens); K/V replicated
within a batch group; host slices inputs / folds weights / gathers outputs.
"""

import os
import sys
import types

import ml_dtypes
import numpy as np

for _p in (
    "/root/.axon_site",
    "/root/.axon_site/_ro/trn_rl_repo",
    "/root/.axon_site/_ro/pypackages",
    "/opt/trn_rl_repo",
):
    if os.path.isdir(_p) and _p not in sys.path:
        sys.path.append(_p)

import concourse.bass as bass  # noqa: E402,F401
import concourse.tile as tile  # noqa: E402
from concourse import bacc, mybir  # noqa: E402
from concourse.bass_utils import run_bass_kernel_spmd  # noqa: E402

F32 = mybir.dt.float32
F32R = mybir.dt.float32r
BF = mybir.dt.bfloat16
Act = mybir.ActivationFunctionType
Alu = mybir.AluOpType
BF_NP = ml_dtypes.bfloat16

B, C, CK = 2, 256, 32
N_TOK = 4096
QCH = 1024
SCALE = (256 // 4) ** (-0.5)
N_CORES = 8

CT = C // 128           # 2 c-tiles of 128
KT = N_TOK // 128       # 32 key tiles of 128
N_WARM = int(os.environ.get("KERNEL_WARMUP", "4"))
FILL_COLS = int(os.environ.get("KERNEL_FILL_COLS", "512"))
FILL_PER_STEP = int(os.environ.get("KERNEL_FILL_PER_STEP", "1"))


def _install_ntff_hook():
    try:
        import antenv.axon_hooks  # noqa: F401
        return True
    except ImportError:
        pass
    try:
        import antenv
        mod = types.ModuleType("antenv.axon_hooks")
        _hook = [None]
        mod.set_axon_ntff_profile_hook = lambda h: _hook.__setitem__(0, h)
        mod.get_axon_ntff_profile_hook = lambda: _hook[0]
        sys.modules["antenv.axon_hooks"] = mod
        antenv.axon_hooks = mod
        from trn_agent_boot.trn_boot import _ntff_profile_via_ctypes
        mod.set_axon_ntff_profile_hook(
            _ntff_profile_via_ctypes("/opt/axon/libaxon_pjrt.so")
        )
        return True
    except Exception:
        return False


def _build_program():
    nc = bacc.Bacc(
        "TRN2", target_bir_lowering=False, debug=False, num_devices=N_CORES
    )
    xq = nc.dram_tensor("xq", [128, CT * QCH], BF, kind="ExternalInput").ap()
    xk4 = nc.dram_tensor("xk4", [128, N_TOK // 4], F32, kind="ExternalInput").ap()
    xkt = nc.dram_tensor("xkt", [128, KT * 33], BF, kind="ExternalInput").ap()
    # w4e: G weights (w4, 4-stacked wg) split into bf16 hi+lo halves
    # (cols 0:257 hi incl. bias col 256, cols 257:514 lo) so the G
    # matmul can accumulate hi+lo and recover fp32-accurate wg.
    w4e = nc.dram_tensor("w4e", [128, 514], BF, kind="ExternalInput").ap()
    wus = nc.dram_tensor("wus", [128, C], F32, kind="ExternalInput").ap()
    out = nc.dram_tensor("out", [C, QCH], F32, kind="ExternalOutput").ap()
    out_r = out.rearrange("(t p) q -> p t q", p=128)
    xq_r = xq.rearrange("p (t q) -> p t q", q=QCH)
    xkt_r = xkt.rearrange("p (t f) -> p t f", f=33)

    with tile.TileContext(nc) as tc:
        with tc.tile_pool(name="singles", bufs=1) as singles:
            xq_sb = singles.tile([128, CT, QCH], BF)
            xk4_sb = singles.tile([128, N_TOK // 4], F32R)
            xkt_sb = singles.tile([128, KT, 33], BF)
            w4e_sb = singles.tile([128, 514], BF)
            wus_sb = singles.tile([128, C], F32R)
            sel_f = singles.tile([128, 128], F32)
            sel_sb = singles.tile([128, 128], F32R)
            zero_f = singles.tile([128, 64], F32)
            zero_sb = singles.tile([128, 64], BF)
            dum = singles.tile([128, 4], F32)
            g4a = singles.tile([128, 512], F32R)
            g4b = singles.tile([128, 512], F32R)

            # Input DMAs dispatched in parallel from three engines, each
            # queue ordered by first use.
            nc.sync.dma_start(out=xq_sb[:, 0, 0:512], in_=xq_r[:, 0, 0:512])
            nc.scalar.dma_start(out=w4e_sb, in_=w4e)
            nc.scalar.dma_start(out=xq_sb[:, 1, 0:512], in_=xq_r[:, 1, 0:512])
            gdma = nc.gpsimd.dma_start
            gdma(out=xk4_sb[:, 0:512], in_=xk4.bitcast(F32R)[:, 0:512])
            gdma(out=xkt_sb[:, 0:16, :], in_=xkt_r[:, 0:16, :])
            gdma(out=xk4_sb[:, 512:1024], in_=xk4.bitcast(F32R)[:, 512:1024])
            gdma(out=xkt_sb[:, 16:32, :], in_=xkt_r[:, 16:32, :])
            gdma(out=wus_sb, in_=wus.bitcast(F32R))
            gdma(out=xq_sb[:, :, 512:1024], in_=xq_r[:, :, 512:1024])

            # sel: row 32 ones -> select-matmul broadcasts Z row 32 (the
            # softmax denominator) across all 128 partitions.
            nc.vector.memset(sel_f, 0.0)
            nc.vector.memset(sel_f[32:33, :], 1.0)
            nc.vector.memset(zero_f, 0.0)
            nc.vector.tensor_copy(sel_sb, sel_f)
            nc.vector.tensor_copy(zero_sb, zero_f)
            # preload the ACT function table during the DMA-in window
            nc.scalar.activation(out=dum[:, 0:1], in_=sel_f[:, 0:1],
                                 func=Act.Exp)

            # PSUM pools: S double-buffer 4 banks, Z/d-broadcast 2
            # (shared tag, alternating slots), y 2 (the y pool also
            # hosts the transient G tiles and the warmup target).
            import contextlib
            psum_stack = contextlib.ExitStack()
            sps = psum_stack.enter_context(
                tc.tile_pool(name="s_ps", bufs=2, space="PSUM"))
            zps = psum_stack.enter_context(
                tc.tile_pool(name="z_ps", bufs=2, space="PSUM"))
            yps = psum_stack.enter_context(
                tc.tile_pool(name="y_ps", bufs=2, space="PSUM"))

            # PE warmup burst: big (256-col) matmuls gated on the w4e
            # arrival bridge the gap until the G matmuls.
            selr = sel_sb
            if N_WARM:
                wm = yps.tile([128, 512], F32, tag="y", name="wm")
                for _ in range(N_WARM):
                    nc.tensor.matmul(
                        wm[0:64, 0:256], lhsT=zero_sb,
                        rhs=w4e_sb[:, 0:256], start=True,
                        stop=True, skip_group_check=True,
                    )

            # ---- G4 = W4.T @ xq + bg4 (4-stacked G for row-tiled S) ----
            g4_t = (g4a, g4b)
            for qi in range(2):
                qsl = slice(qi * 512, (qi + 1) * 512)
                gp = yps.tile([128, 512], F32, tag="y", name="gp")
                for hl in range(2):
                    for ci in range(CT):
                        nc.tensor.matmul(
                            gp,
                            lhsT=w4e_sb[:, hl * 257 + ci * 128:
                                        hl * 257 + (ci + 1) * 128],
                            rhs=xq_sb[:, ci, qsl],
                            start=(hl == 0 and ci == 0),
                            stop=(hl == 1 and ci == CT - 1),
                        )
                nc.scalar.activation(
                    out=g4_t[qi], in_=gp, func=Act.Identity,
                    bias=w4e_sb[:, 256:257], scale=1.0,
                )

            # ---- attention: flat software-pipelined (qi, kp) stream ----
            # Steady state is ACT-bound (exp ~1.15us/step); PE does
            # 2 S + 2 Z matmuls + a zero filler (~1.1us/step).
            steps = [(qi, kp) for qi in range(2) for kp in range(KT // 2)]
            with tc.tile_pool(name="epool", bufs=4) as epool, \
                 tc.tile_pool(name="epi", bufs=2) as epi:
                e_tiles = {}

                def emit_s(i):
                    qi, kp = steps[i]
                    st = sps.tile([128, 1024], F32, tag="s", name="st")
                    for h in range(2):
                        ki = 2 * kp + h
                        jj = ki % 4
                        kk = ki // 4
                        nc.tensor.matmul(
                            st[:, h * 512:(h + 1) * 512],
                            lhsT=xk4_sb[32 * jj:32 * (jj + 1),
                                        128 * kk:128 * (kk + 1)],
                            rhs=g4_t[qi][32 * jj:32 * (jj + 1), :],
                            start=True, stop=True,
                            tile_position=(32 * jj, 0),
                            skip_group_check=True,
                        )
                    e = epool.tile([128, 1024], BF, tag="e", name="e")
                    nc.scalar.activation(out=e, in_=st, func=Act.Exp)
                    e_tiles[i] = e

                def filler(z, n):
                    # Zero-weight matmuls accumulating +0.0 into the live
                    # z group: pure PE occupancy (keeps the HAM duty cycle
                    # at full rate) with no extra PSUM bank and no race.
                    fc = min(FILL_COLS, 512)
                    for _ in range(n):
                        nc.tensor.matmul(
                            z[0:33, 0:fc],
                            lhsT=zero_sb[:, 0:33],
                            rhs=xq_sb[:, 0, 0:fc],
                            start=False, stop=False, skip_group_check=True,
                        )

                z = None
                emitted = 2
                def do_epi_a(qi, zsb):
                    # epilogue stage a: dbc = broadcast d (select-matmul
                    # on sel row 32); rd = 1/dbc on DVE.
                    dbc = zps.tile([128, 512], F32, tag="z", name="dbc")
                    nc.tensor.matmul(
                        dbc, lhsT=selr[0:33, :], rhs=zsb[0:33, :],
                        start=True, stop=True, skip_group_check=True,
                    )
                    rd = epi.tile([128, 512], F32, tag="rd")
                    scr = epi.tile([128, 512], F32, tag="scr")
                    nc.vector.reciprocal_approx_accurate(
                        out=rd, in_=dbc, scratch=scr)
                    return rd

                def do_epi_b(qi, zsb, rd):
                    # epilogue stage b: y = wus[0:33].T @ zsb (includes
                    # boe*d via row 32); out = y*rd + xq.
                    ys = []
                    for co in range(CT):
                        y = yps.tile([128, 512], F32, tag="y")
                        nc.tensor.matmul(
                            y, lhsT=wus_sb[0:33, 128 * co:128 * (co + 1)],
                            rhs=zsb[0:33, :],
                            start=True, stop=True, skip_group_check=True,
                        )
                        ys.append(y)
                    qs = slice(qi * 512, (qi + 1) * 512)
                    if qi == 0:
                        dma_engs = (nc.sync, nc.gpsimd) * 2
                    else:
                        dma_engs = (nc.sync, nc.scalar,
                                    nc.gpsimd, nc.sync)
                    for co in range(CT):
                        t = epi.tile([128, 512], F32, tag="t", bufs=2,
                                     name=f"t{co}")
                        nc.vector.tensor_mul(t, ys[co], rd)
                        add_eng = (nc.gpsimd if qi == 1 and co == 0
                                   else nc.vector)
                        add_eng.tensor_add(t, t, xq_sb[:, co, qs])
                        for hf in range(2):
                            fs = slice(hf * 256, (hf + 1) * 256)
                            qfs = slice(qi * 512 + hf * 256,
                                        qi * 512 + (hf + 1) * 256)
                            eng = dma_engs[co * 2 + hf]
                            eng.dma_start(
                                out=out_r[:, co, qfs], in_=t[:, fs])

                pending_epi = []
                epi_body = -10
                emit_s(0)
                emit_s(1)
                for i, (qi, kp) in enumerate(steps):
                    last_kp = KT // 2 - 1
                    if kp == 0:
                        z = zps.tile([128, 512], F32, tag="z", name="z")
                    e = e_tiles.pop(i)
                    # S for step i+2 first: its gate (exp(i) freeing the
                    # sps slot) fires with e(i), and the next exp waits
                    # on it -- emitting it before the Z pair takes it
                    # off the critical path.  At an epilogue step also
                    # pre-emit i+3 so the epilogue matmuls don't delay it.
                    while emitted <= i + 2 and emitted < len(steps):
                        emit_s(emitted)
                        emitted += 1
                    for h in range(2):
                        nc.tensor.matmul(
                            z[0:33, :],
                            lhsT=xkt_sb[:, 2 * kp + h, :],
                            rhs=e[:, h * 512:(h + 1) * 512],
                            start=(kp == 0 and h == 0),
                            stop=(kp == last_kp and h == 1),
                            skip_group_check=True,
                        )
                    while pending_epi and pending_epi[0][0] <= i:
                        ent = pending_epi.pop(0)
                        if len(ent) == 3:
                            _, eqi, ezsb = ent
                            erd = do_epi_a(eqi, ezsb)
                            pending_epi.append((i + 1, eqi, ezsb, erd))
                        else:
                            _, eqi, ezsb, erd = ent
                            do_epi_b(eqi, ezsb, erd)
                        epi_body = i
                    if (FILL_PER_STEP and kp < last_kp
                            and i > epi_body + 1):
                        filler(z, FILL_PER_STEP)
                    if kp == last_kp and emitted < len(steps):
                        # pre-emit one extra S so the epilogue matmuls
                        # below don't delay it (it gates exp(i+3)); after
                        # the Z pair so its wait can't block them.
                        emit_s(emitted)
                        emitted += 1
                    if kp == last_kp:
                        # epilogue part 1: cast Z out of PSUM right away
                        # (DVE, runs as soon as the Z group stops); the
                        # matmul half is deferred two bodies so its
                        # cast-gated instructions don't stall the
                        # in-order PE queue behind the next steps' S
                        # pairs (and the next z gets a fresh bank).
                        zsb = epi.tile([128, 512], F32R, tag="zsb")
                        nc.vector.tensor_copy(zsb[0:33, :], z[0:33, :])
                        pending_epi.append(
                            (min(i + 2, len(steps) - 1), qi, zsb))
                    # second pop site: the tail epilogue (appended just
                    # above at the final body) must still run this body
                    while pending_epi and (pending_epi[0][0] <= i
                                           or i == len(steps) - 1):
                        ent = pending_epi.pop(0)
                        if len(ent) == 3:
                            _, eqi, ezsb = ent
                            erd = do_epi_a(eqi, ezsb)
                            if i >= len(steps) - 1:
                                do_epi_b(eqi, ezsb, erd)
                            else:
                                pending_epi.append((i + 1, eqi, ezsb, erd))
                        else:
                            _, eqi, ezsb, erd = ent
                            do_epi_b(eqi, ezsb, erd)
                        epi_body = i
            psum_stack.close()

    nc.compile()
    return nc


_NC = None


def _get_nc():
    global _NC
    if _NC is None:
        _NC = _build_program()
    return _NC


def kernel(F_VNet, F_Knowledge, Wq, bq, Wk, bk, Wv, bv, Wo, bo):
    F_VNet = np.asarray(F_VNet, dtype=np.float32)
    F_Knowledge = np.asarray(F_Knowledge, dtype=np.float32)
    Wq, bq = np.asarray(Wq, np.float32), np.asarray(bq, np.float32)
    Wv, bv = np.asarray(Wv, np.float32), np.asarray(bv, np.float32)
    Wk = np.asarray(Wk, np.float32)
    Wo, bo = np.asarray(Wo, np.float32), np.asarray(bo, np.float32)

    in_shape = F_VNet.shape
    xq_full = F_VNet.reshape(B, C, N_TOK)
    xk_full = F_Knowledge.reshape(B, CK, N_TOK)

    wg_h = (SCALE * Wq.T.astype(np.float64) @ Wk.astype(np.float64)).astype(np.float32)
    wu_h = (Wv.T.astype(np.float64) @ Wo.T.astype(np.float64)).astype(np.float32)
    bg_h = (SCALE * (Wk.T @ bq)).astype(np.float32)
    boe_h = (bo + Wo @ bv).astype(np.float32)

    w4_h = np.tile(wg_h, (1, 4))                                # [256, 128]
    bg4_h = np.tile(bg_h, 4)                                    # [128]
    # w4e[p, t*128+f] = w4[t*128+p, f]; col 256 = bg4[p]; bf16 hi in
    # cols 0:257, bf16 lo (fp32 remainder) in cols 257:514
    w4f = np.zeros((128, 257), np.float32)
    for t in range(CT):
        w4f[:, t * 128:(t + 1) * 128] = w4_h[t * 128:(t + 1) * 128, :]
    w4f[:, 256] = bg4_h
    w4hi = w4f.astype(BF_NP)
    w4lo = (w4f - w4hi.astype(np.float32)).astype(BF_NP)
    w4lo[:, 256] = 0
    w4e_h = np.ascontiguousarray(np.concatenate([w4hi, w4lo], axis=1))
    wus_h = np.zeros((128, C), np.float32)
    wus_h[0:32] = wu_h
    wus_h[32] = boe_h          # y row: + boe*d  ->  y/d = attn_out + boe

    # per-batch key layouts
    xk4_b, xkt_b = [], []
    for b in range(B):
        xk = xk_full[b]                                          # [32, 4096]
        xk4_b.append(np.ascontiguousarray(
            xk.reshape(32, 8, 4, 128).transpose(2, 0, 1, 3).reshape(128, 1024)
        ))
        xkT33 = np.concatenate(
            [xk.T, np.ones((N_TOK, 1), np.float32)], axis=1)     # [4096, 33]
        xkt_b.append(np.ascontiguousarray(
            xkT33.reshape(KT, 128, 33).transpose(1, 0, 2).reshape(128, KT * 33)
        ).astype(BF_NP))

    in_maps = []
    for core in range(N_CORES):
        b, j = divmod(core, N_CORES // B)
        xq_img = np.ascontiguousarray(
            xq_full[b, :, j * QCH:(j + 1) * QCH]
            .reshape(CT, 128, QCH).transpose(1, 0, 2).reshape(128, CT * QCH))
        in_maps.append({
            "xq": xq_img.astype(BF_NP),
            "xk4": xk4_b[b], "xkt": xkt_b[b],
            "w4e": w4e_h, "wus": wus_h,
        })

    trace = bool(os.environ.get("KERNEL_TRACE"))
    if trace:
        _install_ntff_hook()
    nc = _get_nc()
    res = run_bass_kernel_spmd(
        nc, in_maps, core_ids=list(range(N_CORES)), trace=trace
    )
    kernel.last_results = res

    out = np.empty((B, C, N_TOK), np.float32)
    for core in range(N_CORES):
        b, j = divmod(core, N_CORES // B)
        out[b, :, j * QCH:(j + 1) * QCH] = res.results[core]["out"]
    return out.reshape(in_shape)
